# revision 1
# baseline (speedup 1.0000x reference)
"""CrossAttention Trainium2 kernel (8 NeuronCores).

Reference computation (B=2, N=M=2048, D=1024, H=16, C=64):
    q = rmsnorm(querys @ Wq.T, gq) * C**-0.5       [B,N,D]
    k = rmsnorm(key_feats @ Wk.T, gk)              [B,M,D]
    v = key_feats @ Wv.T                           [B,M,D]
    attn = softmax(mask(q @ k.T per head))         [B,H,N,M]
    out = (attn @ v per head, concat) @ Wo.T + bo  [B,N,D]

Sharding: core = b*4 + j (b in {0,1}; j in {0..3} owns heads 4j..4j+3 = a
256-wide slice of D). Host pre-transposes inputs/weights, folds gq*scale /
gk into Wq / Wk rows, and pre-rounds everything to f32r (fp32 with 11-bit
mantissa -> full PE rate). Per core:

  - q'^T / k'^T projections in d-slice layout [256, 2048] (contraction over
    E in the partition dim), v in [2048, 256]. q' = gs_q * q_raw etc.
  - rmsnorm sum-of-squares over the FULL D: per-core partial sumsq is
    computed by a matmul against a 1/gs^2-weighted column (compensating the
    folded gains) and AllReduced (8KB) across the 4 cores of each b; the
    collectives are emitted right after their producing phase so they hide
    behind the next projection.
  - rstd chains run lane-parallel in [128,16] layout. rstd_k is NOT applied
    to k': in the S^T = k'q'^T orientation the softmax logit scale rstd_k[m]
    is per-partition, so it folds into the exp ACTIVATE as its scale operand
    (and the mask as its bias: 0 / -1e30). rstd_q is applied to q' via a
    PE-transpose into row layout + ones outer-product broadcast.
  - attention per head: for each m-tile, 4 QK matmuls (one per 512-wide
    n-block, shared k stationary) -> batched exp -> 4 PV matmuls into a
    4-bank accumulator. v carries a 65th column of ones so row 64 of the
    accumulator is the softmax denominator (reciprocal_approx_fast + ones
    outer-product broadcast + one multiply normalizes the head output).
  - out projection produces a partial out^T [1024, 2048] (contraction over
    this core's d-slice only); the host sums 4 partials per b and adds bo.
"""

import os

import numpy as np

import concourse.tile as tile
from concourse import bacc, mybir
from concourse.bass_utils import run_bass_kernel_spmd

DEBUG = bool(os.environ.get("BASSK_DEBUG"))

B, N, M, D, H = 2, 2048, 2048, 1024, 16
C = D // H  # 64, head dim
E = D  # input feature dim
EPS = 1e-6
SCALE = C ** (-0.5)
DS = D // 4  # 256, per-core d-slice
NCORES = 8

f32 = mybir.dt.float32
f32r = mybir.dt.float32r
AF = mybir.ActivationFunctionType

NEG = -1e30


def round_f32r(x: np.ndarray) -> np.ndarray:
    b = np.ascontiguousarray(x, dtype=np.float32).view(np.uint32)
    b = (b + 0x800) & np.uint32(0xFFFFF000)
    return b.view(np.float32)


def build():
    nc = bacc.Bacc(None, target_bir_lowering=False)

    qT_d = nc.declare_dram_parameter("qT", [E, N], f32r, isOutput=False)
    kfT_d = nc.declare_dram_parameter("kfT", [E, M], f32r, isOutput=False)
    wqT_d = nc.declare_dram_parameter("wqT", [E, DS], f32r, isOutput=False)
    wkT_d = nc.declare_dram_parameter("wkT", [E, DS], f32r, isOutput=False)
    wvT_d = nc.declare_dram_parameter("wvT", [E, DS], f32r, isOutput=False)
    woT_d = nc.declare_dram_parameter("woT", [DS, D], f32r, isOutput=False)
    ig2q_d = nc.declare_dram_parameter("ig2q", [2, 128], f32r, isOutput=False)
    ig2k_d = nc.declare_dram_parameter("ig2k", [2, 128], f32r, isOutput=False)
    mb_d = nc.declare_dram_parameter("mbias", [16, 128], f32, isOutput=False)
    outT_d = nc.declare_dram_parameter("outT", [D, N], f32, isOutput=True)
    if DEBUG:
        dbg_q = nc.declare_dram_parameter("dbg_q", [128, 2, 4, 512], f32r, isOutput=True)
        dbg_k = nc.declare_dram_parameter("dbg_k", [128, 2, 4, 512], f32r, isOutput=True)
        dbg_v = nc.declare_dram_parameter("dbg_v", [128, 16, 4, C + 1], f32r, isOutput=True)
        dbg_x = nc.declare_dram_parameter("dbg_x", [128, 2, 4, 512], f32r, isOutput=True)
        dbg_rk = nc.declare_dram_parameter("dbg_rk", [128, 16], f32, isOutput=True)
        dbg_rq = nc.declare_dram_parameter("dbg_rq", [1, 2048], f32, isOutput=True)
        dbg_s = nc.declare_dram_parameter("dbg_s", [128, 2, 512], f32, isOutput=True)
        dbg_p = nc.declare_dram_parameter("dbg_p", [128, 2, 512], f32r, isOutput=True)
        dbg_o = nc.declare_dram_parameter("dbg_o", [C + 1, 2, 512], f32, isOutput=True)
        dbg_rd = nc.declare_dram_parameter("dbg_rd", [1, 512], f32, isOutput=True)
        dbg_bc = nc.declare_dram_parameter("dbg_bc", [C, 512], f32, isOutput=True)

    with (
        nc.allow_low_precision(reason="f32r matmul operands by design; fp32 PSUM"),
        tile.TileContext(nc) as tc,
    ):
        with (
            tc.tile_pool(name="singles", bufs=1) as singles,
            tc.tile_pool(name="wts", bufs=2) as wts,
            tc.tile_pool(name="blk", bufs=1 if DEBUG else 2) as blkpool,
            tc.tile_pool(name="sq", bufs=2) as sqpool,
            tc.tile_pool(name="psb", bufs=3) as ppool,
            tc.tile_pool(name="obuf", bufs=2) as obuf,
            tc.tile_pool(name="rdp", bufs=8) as rdp,
            tc.tile_pool(name="small", bufs=2) as small,
            tc.tile_pool(name="dram", bufs=1, space="DRAM") as dram,
        ):
            # ---- constants / small inputs ----
            ones_f = singles.tile([128, 64], f32)
            nc.vector.memset(ones_f, 1.0)
            ones1x64 = singles.tile([1, 64], f32)
            nc.vector.memset(ones1x64, 1.0)
            ones1x128 = singles.tile([1, 128], f32)
            nc.vector.memset(ones1x128, 1.0)
            eps_t = singles.tile([128, 1], f32)
            nc.vector.memset(eps_t, EPS)
            invd_t = singles.tile([128, 1], f32)
            nc.vector.memset(invd_t, 1.0 / D)
            ig2q_sb = singles.tile([128, 2], f32r)
            nc.sync.dma_start(out=ig2q_sb, in_=ig2q_d.rearrange("t p -> p t"))
            ig2k_sb = singles.tile([128, 2], f32r)
            nc.sync.dma_start(out=ig2k_sb, in_=ig2k_d.rearrange("t p -> p t"))
            mb_sb = singles.tile([128, 16], f32)
            nc.sync.dma_start(out=mb_sb, in_=mb_d.rearrange("t p -> p t"))

            # weights rotate through 2 pool slots: wq,wk up front; wv,wo reuse
            wq_sb = wts.tile([128, 8, DS], f32r, tag="w")
            wk_sb = wts.tile([128, 8, DS], f32r, tag="w")
            for et in range(8):
                nc.sync.dma_start(out=wq_sb[:, et, :], in_=wqT_d[et * 128 : et * 128 + 128, :])
                nc.scalar.dma_start(out=wk_sb[:, et, :], in_=wkT_d[et * 128 : et * 128 + 128, :])

            # ---- persistent activations ----
            qT = singles.tile([128, 2, 4, 512], f32r)  # [p, dt, nb, n]
            kT = singles.tile([128, 2, 4, 512], f32r)  # [p, dt, mb, m]
            v_sb = singles.tile([128, 16, 4, C + 1], f32r)  # [m_p, mt, h, c|ones]
            xT = singles.tile([128, 2, 4, 512], f32r)  # [p, dt, nb, n]
            nc.vector.tensor_copy(
                v_sb[:, :, :, C], ones_f.rearrange("p (a b) -> p a b", a=16)
            )

            ccq_in = dram.tile([2048], f32)
            ccq_out = dram.tile([2048], f32)
            cck_in = dram.tile([2048], f32)
            cck_out = dram.tile([2048], f32)

            def projection(src_d, w_sb, dst, ig2_sb, cc_in_t, dma_eng):
                """dst[dt, nb] = W'^T-slice @ src-block; partial sumsq -> cc_in."""
                for nb in range(4):
                    blk = blkpool.tile([128, 8, 512], f32r, tag="blk")
                    for et in range(8):
                        dma_eng.dma_start(
                            out=blk[:, et, :],
                            in_=src_d[et * 128 : et * 128 + 128, nb * 512 : nb * 512 + 512],
                        )
                    ss_ps = ssps.tile([1, 512], f32, tag="ss")
                    for dt in range(2):
                        ps = projps.tile([128, 512], f32, tag="proj")
                        for et in range(8):
                            nc.tensor.matmul(
                                ps,
                                w_sb[:, et, dt * 128 : dt * 128 + 128],
                                blk[:, et, :],
                                start=(et == 0),
                                stop=(et == 7),
                            )
                        nc.vector.tensor_copy(dst[:, dt, nb, :], ps)
                        sq = sqpool.tile([128, 512], f32r, tag="sq")
                        nc.vector.tensor_mul(sq, dst[:, dt, nb, :], dst[:, dt, nb, :])
                        nc.tensor.matmul(
                            ss_ps,
                            ig2_sb[:, dt : dt + 1],
                            sq,
                            start=(dt == 0),
                            stop=(dt == 1),
                            skip_group_check=True,
                        )
                    ss_sb = small.tile([1, 512], f32, tag="ss_sb")
                    nc.scalar.copy(ss_sb, ss_ps)
                    nc.sync.dma_start(
                        out=cc_in_t[nb * 512 : nb * 512 + 512].rearrange(
                            "(a n) -> a n", a=1
                        ),
                        in_=ss_sb,
                    )

            def rstd128(cc_out_t, tag):
                """[128,16] lane-parallel rstd chain: p,t -> 1/sqrt(ss/D+eps)."""
                ss128 = small.tile([128, 16], f32, tag=f"ss128{tag}")
                nc.sync.dma_start(
                    out=ss128, in_=cc_out_t.rearrange("(t p) -> p t", p=128)
                )
                std = small.tile([128, 16], f32, tag=f"std{tag}")
                nc.scalar.activation(std, ss128, AF.Sqrt, bias=eps_t, scale=invd_t)
                r = singles.tile([128, 16], f32)
                nc.vector.reciprocal_approx_fast(out=r, in_=std)
                return r

            with (
                tc.tile_pool(name="projps", bufs=2, space="PSUM") as projps,
                tc.tile_pool(name="vps", bufs=2, space="PSUM") as vps,
                tc.tile_pool(name="ssps", bufs=2, space="PSUM") as ssps,
            ):
                # ---- q projection, then its collective (hidden behind k/v) ----
                projection(qT_d, wq_sb, qT, ig2q_sb, ccq_in, nc.sync)
                nc.gpsimd.collective_compute(
                    "AllReduce",
                    mybir.AluOpType.add,
                    replica_groups=[[0, 1, 2, 3], [4, 5, 6, 7]],
                    ins=[ccq_in.opt()],
                    outs=[ccq_out.opt()],
                )

                # ---- k projection, then its collective (hidden behind v) ----
                projection(kfT_d, wk_sb, kT, ig2k_sb, cck_in, nc.scalar)
                nc.gpsimd.collective_compute(
                    "AllReduce",
                    mybir.AluOpType.add,
                    replica_groups=[[0, 1, 2, 3], [4, 5, 6, 7]],
                    ins=[cck_in.opt()],
                    outs=[cck_out.opt()],
                )

                # ---- v projection (kfT re-streamed) ----
                wv_sb = wts.tile([128, 8, DS], f32r, tag="w")
                for et in range(8):
                    nc.scalar.dma_start(out=wv_sb[:, et, :], in_=wvT_d[et * 128 : et * 128 + 128, :])
                for mb in range(4):
                    blk = blkpool.tile([128, 8, 512], f32r, tag="blk")
                    for et in range(8):
                        nc.scalar.dma_start(
                            out=blk[:, et, :],
                            in_=kfT_d[et * 128 : et * 128 + 128, mb * 512 : mb * 512 + 512],
                        )
                    for mt in range(4):
                        psv = vps.tile([128, 256], f32, tag="v")
                        for et in range(8):
                            nc.tensor.matmul(
                                psv,
                                blk[:, et, mt * 128 : mt * 128 + 128],
                                wv_sb[:, et, :],
                                start=(et == 0),
                                stop=(et == 7),
                            )
                        nc.vector.tensor_copy(
                            v_sb[:, mb * 4 + mt, :, 0:C],
                            psv.rearrange("p (h c) -> p h c", c=C),
                        )

                # ---- rstd_k: [128,16] lane-parallel; feeds exp scale directly ----
                rstdk = rstd128(cck_out, "k")

                # ---- rstd_q: row layout [1, 2048] for the bcast outer-products ----
                ssq_row = singles.tile([1, 2048], f32)
                nc.sync.dma_start(
                    out=ssq_row, in_=ccq_out.rearrange("(a n) -> a n", a=1)
                )
                nc.scalar.activation(
                    ssq_row, ssq_row, AF.Sqrt, bias=eps_t[0:1, :], scale=invd_t[0:1, :]
                )
                rs_row = singles.tile([1, 2048], f32)
                nc.vector.reciprocal_approx_fast(out=rs_row, in_=ssq_row)
                # q finalize: qT[d, n] *= rstd_q[n] via ones outer-product bcast
                for nb in range(4):
                    bcq = projps.tile([128, 512], f32, tag="proj")
                    nc.tensor.matmul(
                        bcq,
                        ones1x128,
                        rs_row[:, nb * 512 : nb * 512 + 512],
                        start=True,
                        stop=True,
                    )
                    for dt in range(2):
                        nc.vector.tensor_mul(qT[:, dt, nb, :], qT[:, dt, nb, :], bcq)

                # ---- HAM warm-up burst: ~5us of dense dependency-free matmuls
                # (the collective/norm stall re-throttles the PE clock to 4/8;
                # a fully-busy 3.4us window is needed to flip it back to 8/8
                # before the attention stream, whose fine-grained gaps can
                # never re-warm it)
                warm = projps.tile([128, 512], f32, tag="proj")
                for i in range(20):
                    nc.tensor.matmul(
                        warm,
                        kT[:, 0, 0, 0:128],
                        kT[:, 0, 1, :],
                        start=(i == 0),
                        stop=(i == 19),
                        skip_group_check=True,
                    )
                warm_sink = small.tile([1, 512], f32, tag="rd")
                nc.vector.tensor_copy(warm_sink, warm[0:1, :])

            # ---- phase 3: attention over (head, nb-pair) passes ----
            # PSUM: s2 (2 banks x 2 bufs) + o2 (2 banks) + dummy (1) = 7 banks.
            # A dependency-free dummy matmul per m-tile bridges the PE's
            # ~150ns/mt deficit vs the ACT exp pacing: any recurring PE gap
            # keeps the HAM clock-gate at K=4/8 (half clock), which costs far
            # more than the dummy's 213ns.
            with (
                tc.tile_pool(name="sps", bufs=2, space="PSUM") as spool,
                tc.tile_pool(name="ops", bufs=1, space="PSUM") as opool,
                tc.tile_pool(name="dmy", bufs=1, space="PSUM") as dmypool,
            ):
                dum = dmypool.tile([128, 512], f32, tag="dum")

                def emit_dummy():
                    nc.tensor.matmul(
                        dum, kT[:, 0, 0, 0:128], kT[:, 0, 1, :],
                        start=True, stop=True, skip_group_check=True,
                    )

                def emit_normalize(state):
                    """bc outer-products + muls for a pass whose DVE recips are
                    done by now (emitted one pass late to keep PE gapless)."""
                    hh, nbp, oo_sb, rds = state
                    ddt, ooff = hh // 2, (hh % 2) * C
                    for i, nb in enumerate((2 * nbp, 2 * nbp + 1)):
                        bc = spool.tile([128, 2, 512], f32, tag="s2")
                        nc.tensor.matmul(
                            bc[0:C, 0, :], ones1x64, rds[i], start=True, stop=True
                        )
                        nc.vector.tensor_mul(
                            xT[ooff : ooff + C, ddt, nb, :],
                            oo_sb[0:C, i, :],
                            bc[0:C, 0, :],
                        )

                prev = None
                for h in range(4):
                    dt, off = h // 2, (h % 2) * C
                    for nbp in range(2):
                        nbs = (2 * nbp, 2 * nbp + 1)
                        o2 = opool.tile([C + 1, 2, 512], f32, tag="o2")
                        for mt in range(16):
                            kT_lhs = kT[
                                off : off + C, dt, mt // 4,
                                (mt % 4) * 128 : (mt % 4) * 128 + 128,
                            ]
                            s2 = spool.tile([128, 2, 512], f32, tag="s2")
                            for i, nb in enumerate(nbs):
                                nc.tensor.matmul(
                                    s2[:, i, :],
                                    kT_lhs,
                                    qT[off : off + C, dt, nb, :],
                                    start=True,
                                    stop=True,
                                )
                            p2 = ppool.tile([128, 2, 512], f32r, tag="p")
                            nc.scalar.activation(
                                p2, s2, AF.Exp,
                                bias=mb_sb[:, mt : mt + 1],
                                scale=rstdk[:, mt : mt + 1],
                            )
                            if DEBUG and h == 0 and mt == 0 and nbp == 0:
                                s_sb = ppool.tile([128, 2, 512], f32, tag="dbgs")
                                nc.vector.tensor_copy(s_sb, s2)
                                nc.sync.dma_start(out=dbg_s[:], in_=s_sb)
                                nc.sync.dma_start(out=dbg_p[:], in_=p2)
                            for i in range(2):
                                nc.tensor.matmul(
                                    o2[:, i, :],
                                    v_sb[:, mt, h, :],
                                    p2[:, i, :],
                                    start=(mt == 0),
                                    stop=(mt == 15),
                                    skip_group_check=True,
                                )
                            emit_dummy()
                        # free o2 with a single copy; recips run during the
                        # next pass, bc+mul are emitted one pass late
                        o_sb = obuf.tile([C + 1, 2, 512], f32, tag="osb")
                        nc.vector.tensor_copy(o_sb, o2)
                        if DEBUG and h == 0 and nbp == 0:
                            nc.sync.dma_start(out=dbg_o[:], in_=o_sb)
                        rds = []
                        for i in range(2):
                            den_sb = rdp.tile([1, 512], f32, tag="den")
                            nc.vector.tensor_copy(den_sb, o_sb[C : C + 1, i, :])
                            rd = rdp.tile([1, 512], f32, tag="rd")
                            nc.vector.reciprocal_approx_fast(out=rd, in_=den_sb)
                            rds.append(rd)
                        if prev is not None:
                            emit_normalize(prev)
                        prev = (h, nbp, o_sb, rds)
                emit_normalize(prev)
                warm_sink2 = small.tile([1, 512], f32, tag="ss_sb")
                nc.vector.tensor_copy(warm_sink2, dum[0:1, :])

            if DEBUG:
                nc.sync.dma_start(out=dbg_q[:], in_=qT)
                nc.sync.dma_start(out=dbg_k[:], in_=kT)
                nc.sync.dma_start(out=dbg_v[:], in_=v_sb)
                nc.sync.dma_start(out=dbg_x[:], in_=xT)
                nc.sync.dma_start(out=dbg_rk[:], in_=rstdk)
                nc.sync.dma_start(out=dbg_rq[:], in_=rs_row)

            # ---- phase 4: out projection (partial over d-slice) ----
            with tc.tile_pool(name="outps", bufs=3, space="PSUM") as outps:
                wo_sb = wts.tile([128, 2, D], f32r, tag="w")
                for dc in range(2):
                    nc.sync.dma_start(
                        out=wo_sb[:, dc, :], in_=woT_d[dc * 128 : dc * 128 + 128, :]
                    )
                for nb in range(4):
                    for ot in range(8):
                        ps = outps.tile([128, 512], f32, tag="out")
                        for dc in range(2):
                            nc.tensor.matmul(
                                ps,
                                wo_sb[:, dc, ot * 128 : ot * 128 + 128],
                                xT[:, dc, nb, :],
                                start=(dc == 0),
                                stop=(dc == 1),
                            )
                        out_sb = ppool.tile([128, 512], f32, tag="osb")
                        nc.scalar.copy(out_sb, ps)
                        nc.sync.dma_start(
                            out=outT_d[ot * 128 : ot * 128 + 128, nb * 512 : nb * 512 + 512],
                            in_=out_sb,
                        )

    nc.finalize()
    return nc


_NC_CACHE = None


def _get_nc():
    global _NC_CACHE
    if _NC_CACHE is None:
        _NC_CACHE = build()
    return _NC_CACHE


def make_in_maps(querys, key_feats, mask, Wq, Wk, Wv, gq, gk, Wo, bo):
    querys = np.asarray(querys, dtype=np.float32)
    key_feats = np.asarray(key_feats, dtype=np.float32)
    mask = np.asarray(mask)
    gq = np.asarray(gq, dtype=np.float32)
    gk = np.asarray(gk, dtype=np.float32)

    gsq_full = gq * np.float32(SCALE)  # folded into Wq rows
    gsk_full = gk.astype(np.float32)  # folded into Wk rows
    Wq_f = np.asarray(Wq, dtype=np.float32) * gsq_full[:, None]
    Wk_f = np.asarray(Wk, dtype=np.float32) * gsk_full[:, None]

    qT = [round_f32r(querys[b].T) for b in range(B)]
    kfT = [round_f32r(key_feats[b].T) for b in range(B)]
    mb = [
        np.where(mask[b] == 0, np.float32(NEG), np.float32(0.0))
        .astype(np.float32)
        .reshape(16, 128)
        for b in range(B)
    ]
    wqT, wkT, wvT, woT, ig2q, ig2k = [], [], [], [], [], []
    for j in range(4):
        dsl = slice(j * DS, (j + 1) * DS)
        wqT.append(round_f32r(Wq_f[dsl].T))
        wkT.append(round_f32r(Wk_f[dsl].T))
        wvT.append(round_f32r(np.asarray(Wv)[dsl].T))
        woT.append(round_f32r(np.asarray(Wo)[:, dsl].T))
        # sumsq compensation: raw sumsq = sum_d (q'_d)^2 / gs_d^2
        ig2q.append(round_f32r((1.0 / gsq_full[dsl] ** 2).reshape(2, 128)))
        ig2k.append(round_f32r((1.0 / gsk_full[dsl] ** 2).reshape(2, 128)))

    in_maps = []
    for cid in range(NCORES):
        b, j = cid // 4, cid % 4
        in_maps.append(
            {
                "qT": qT[b],
                "kfT": kfT[b],
                "wqT": wqT[j],
                "wkT": wkT[j],
                "wvT": wvT[j],
                "woT": woT[j],
                "ig2q": ig2q[j],
                "ig2k": ig2k[j],
                "mbias": mb[b],
            }
        )
    return in_maps


def assemble(results, bo):
    bo = np.asarray(bo, dtype=np.float32)
    out = np.zeros((B, N, D), dtype=np.float32)
    for cid in range(NCORES):
        b = cid // 4
        out[b] += results[cid]["outT"].T
    out += bo
    return out


def kernel(querys, key_feats, mask, Wq, Wk, Wv, gq, gk, Wo, bo):
    nc = _get_nc()
    in_maps = make_in_maps(querys, key_feats, mask, Wq, Wk, Wv, gq, gk, Wo, bo)
    res = run_bass_kernel_spmd(nc, in_maps, list(range(NCORES)))
    return assemble(res.results, bo)



# revision 4
# speedup vs baseline: 1.2324x; 1.2324x over previous
"""CrossAttention Trainium2 kernel (8 NeuronCores).

Reference computation (B=2, N=M=2048, D=1024, H=16, C=64):
    q = rmsnorm(querys @ Wq.T, gq) * C**-0.5       [B,N,D]
    k = rmsnorm(key_feats @ Wk.T, gk)              [B,M,D]
    v = key_feats @ Wv.T                           [B,M,D]
    attn = softmax(mask(q @ k.T per head))         [B,H,N,M]
    out = (attn @ v per head, concat) @ Wo.T + bo  [B,N,D]

Sharding: core = b*4 + j (b in {0,1}; j in {0..3} owns heads 4j..4j+3 = a
256-wide slice of D). Host pre-transposes inputs/weights, folds gq*scale /
gk into Wq / Wk rows, and pre-rounds everything to f32r (fp32 with 11-bit
mantissa -> full PE rate).

v2 structural changes vs v1:
  - Mask compaction: rows with mask==0 contribute exp(-inf)=0 to both the
    softmax denominator and PV, so the host gathers only the valid kf
    columns (per batch), pads to a multiple of 128, and the kernel runs
    with M_pad ~= 1152 instead of 2048.  All M-side work (k/v projection,
    QK, exp, PV, kf DMA) shrinks ~2x.  Padding columns carry bias -1e30
    into the exp -> contribute exactly 0.
  - k and v projections fused over a single kfT stream (halves kf DMA).
  - One fused AllReduce carries both q and k partial sum-of-squares
    (2048 + M_pad floats); its ~27us mesh latency is bridged by a long
    dependency-free dummy-matmul burst that keeps the PE HAM clock-gate
    warm (K=8/8) into attention.
  - Attention is software-pipelined depth-2 per (nbp, h) pass: PE order is
    ... PV(mt-1), QK(mt+1), filler, PV(mt) ... so the PE never stalls on
    the ACT exp (v1 stalled ~0.4us every mt, which kept HAM at K=4/8 =
    1.2 GHz for the whole 314us attention phase).  ACT exp (~1.2us/mt) is
    the pace-setter; PE real work is ~1.0us/mt, padded by a dummy matmul
    (first n-half) or an out-projection matmul pair (second n-half).
  - The out projection (partial over this core's d-slice) is interleaved
    into attention as filler work; the host sums 4 partials per b and
    adds bo.
"""

import numpy as np

import concourse.tile as tile
from concourse import bacc, mybir
from concourse.bass_utils import run_bass_kernel_spmd

B, N, M_FULL, D, H = 2, 2048, 2048, 1024, 16
C = D // H  # 64, head dim
E = D  # input feature dim
EPS = 1e-6
SCALE = C ** (-0.5)
DS = D // 4  # 256, per-core d-slice
NCORES = 8

f32 = mybir.dt.float32
f32r = mybir.dt.float32r
AF = mybir.ActivationFunctionType

NEG = -1e30
WARM_MM = 130  # dummy matmuls bridging the AllReduce latency (~27us @ 2.4GHz)


def round_f32r(x: np.ndarray) -> np.ndarray:
    b = np.ascontiguousarray(x, dtype=np.float32).view(np.uint32)
    b = (b + 0x800) & np.uint32(0xFFFFF000)
    return b.view(np.float32)


def build(n_mt: int):
    M = n_mt * 128
    mblocks = []
    off = 0
    while off < M:
        w = min(512, M - off)
        mblocks.append((off, w))
        off += w

    nc = bacc.Bacc(None, target_bir_lowering=False)

    qT_d = nc.declare_dram_parameter("qT", [E, N], f32r, isOutput=False)
    kfT_d = nc.declare_dram_parameter("kfT", [E, M], f32r, isOutput=False)
    wqT_d = nc.declare_dram_parameter("wqT", [E, DS], f32r, isOutput=False)
    wkT_d = nc.declare_dram_parameter("wkT", [E, DS], f32r, isOutput=False)
    wvT_d = nc.declare_dram_parameter("wvT", [E, DS], f32r, isOutput=False)
    woT_d = nc.declare_dram_parameter("woT", [DS, D], f32r, isOutput=False)
    ig2q_d = nc.declare_dram_parameter("ig2q", [2, 128], f32r, isOutput=False)
    ig2k_d = nc.declare_dram_parameter("ig2k", [2, 128], f32r, isOutput=False)
    mb_d = nc.declare_dram_parameter("mbias", [n_mt, 128], f32, isOutput=False)
    outT_d = nc.declare_dram_parameter("outT", [D, N], f32, isOutput=True)

    with (
        nc.allow_low_precision(reason="f32r matmul operands by design; fp32 PSUM"),
        tile.TileContext(nc) as tc,
    ):
        with (
            tc.tile_pool(name="singles", bufs=1) as singles,
            tc.tile_pool(name="wts", bufs=3) as wts,
            tc.tile_pool(name="blk", bufs=2) as blkpool,
            tc.tile_pool(name="sq", bufs=2) as sqpool,
            tc.tile_pool(name="psb", bufs=3) as ppool,
            tc.tile_pool(name="obuf", bufs=2) as obuf,
            tc.tile_pool(name="osb2", bufs=2) as outbuf,
            tc.tile_pool(name="rdp", bufs=6) as rdp,
            tc.tile_pool(name="small", bufs=2) as small,
            tc.tile_pool(name="dram", bufs=1, space="DRAM") as dram,
        ):
            # ---- constants / small inputs ----
            ones1x64 = singles.tile([1, 64], f32)
            nc.vector.memset(ones1x64, 1.0)
            ones1x128 = singles.tile([1, 128], f32)
            nc.vector.memset(ones1x128, 1.0)
            onesv = singles.tile([128, n_mt * 4], f32)
            nc.vector.memset(onesv, 1.0)
            eps_t = singles.tile([128, 1], f32)
            nc.vector.memset(eps_t, EPS)
            invd_t = singles.tile([128, 1], f32)
            nc.vector.memset(invd_t, 1.0 / D)
            ig2q_sb = singles.tile([128, 2], f32r)
            nc.sync.dma_start(out=ig2q_sb, in_=ig2q_d.rearrange("t p -> p t"))
            ig2k_sb = singles.tile([128, 2], f32r)
            nc.sync.dma_start(out=ig2k_sb, in_=ig2k_d.rearrange("t p -> p t"))
            mb_sb = singles.tile([128, n_mt], f32)
            nc.sync.dma_start(out=mb_sb, in_=mb_d.rearrange("t p -> p t"))

            # weights: wq, wk, wv upfront; wo reuses wq's slot after q proj
            wq_sb = wts.tile([128, 8, DS], f32r, tag="w")
            wk_sb = wts.tile([128, 8, DS], f32r, tag="w")
            wv_sb = wts.tile([128, 8, DS], f32r, tag="w")
            for et in range(8):
                nc.sync.dma_start(out=wq_sb[:, et, :], in_=wqT_d[et * 128 : et * 128 + 128, :])
                nc.scalar.dma_start(out=wk_sb[:, et, :], in_=wkT_d[et * 128 : et * 128 + 128, :])
                nc.gpsimd.dma_start(out=wv_sb[:, et, :], in_=wvT_d[et * 128 : et * 128 + 128, :])

            # ---- persistent activations ----
            qT = singles.tile([128, 2, 4, 512], f32r)  # [p, dt, nb, n]
            kT = singles.tile([128, 2, M], f32r)  # [p, dt, m]
            v_sb = singles.tile([128, n_mt, 4, C + 1], f32r)  # [m_p, mt, h, c|ones]
            xT = singles.tile([128, 2, 4, 512], f32r)  # [p, dt, nb, n]
            nc.vector.tensor_copy(
                v_sb[:, :, :, C], onesv.rearrange("p (a b) -> p a b", a=n_mt)
            )

            cc_in = dram.tile([2048 + M], f32)
            cc_out = dram.tile([2048 + M], f32)

            with (
                tc.tile_pool(name="projps", bufs=2, space="PSUM") as projps,
                tc.tile_pool(name="vps", bufs=2, space="PSUM") as vps,
                tc.tile_pool(name="ssps", bufs=2, space="PSUM") as ssps,
            ):
                # ---- q projection: qT[dt, nb] = Wq'^T-slice @ q-block ----
                for nb in range(4):
                    blk = blkpool.tile([128, 8, 512], f32r, tag="blk")
                    for et in range(8):
                        nc.sync.dma_start(
                            out=blk[:, et, :],
                            in_=qT_d[et * 128 : et * 128 + 128, nb * 512 : nb * 512 + 512],
                        )
                    ss_ps = ssps.tile([1, 512], f32, tag="ss")
                    for dt in range(2):
                        ps = projps.tile([128, 512], f32, tag="proj")
                        for et in range(8):
                            nc.tensor.matmul(
                                ps,
                                wq_sb[:, et, dt * 128 : dt * 128 + 128],
                                blk[:, et, :],
                                start=(et == 0),
                                stop=(et == 7),
                            )
                        nc.vector.tensor_copy(qT[:, dt, nb, :], ps)
                        sq = sqpool.tile([128, 512], f32r, tag="sq")
                        nc.vector.tensor_mul(sq, qT[:, dt, nb, :], qT[:, dt, nb, :])
                        nc.tensor.matmul(
                            ss_ps,
                            ig2q_sb[:, dt : dt + 1],
                            sq,
                            start=(dt == 0),
                            stop=(dt == 1),
                            skip_group_check=True,
                        )
                    ss_sb = small.tile([1, 512], f32, tag="ss_sb")
                    nc.scalar.copy(ss_sb, ss_ps)
                    nc.sync.dma_start(
                        out=cc_in[nb * 512 : nb * 512 + 512].rearrange(
                            "(a n) -> a n", a=1
                        ),
                        in_=ss_sb,
                    )

                # ---- fused k+v projection over one kfT stream ----
                for moff, w in mblocks:
                    blk = blkpool.tile([128, 8, 512], f32r, tag="blk")
                    for et in range(8):
                        nc.scalar.dma_start(
                            out=blk[:, et, 0:w],
                            in_=kfT_d[et * 128 : et * 128 + 128, moff : moff + w],
                        )
                    ss_ps = ssps.tile([1, 512], f32, tag="ss")
                    for dt in range(2):
                        ps = projps.tile([128, 512], f32, tag="proj")
                        for et in range(8):
                            nc.tensor.matmul(
                                ps[:, 0:w],
                                wk_sb[:, et, dt * 128 : dt * 128 + 128],
                                blk[:, et, 0:w],
                                start=(et == 0),
                                stop=(et == 7),
                            )
                        nc.vector.tensor_copy(kT[:, dt, moff : moff + w], ps[:, 0:w])
                        sq = sqpool.tile([128, 512], f32r, tag="sq")
                        nc.vector.tensor_mul(
                            sq[:, 0:w],
                            kT[:, dt, moff : moff + w],
                            kT[:, dt, moff : moff + w],
                        )
                        nc.tensor.matmul(
                            ss_ps[:, 0:w],
                            ig2k_sb[:, dt : dt + 1],
                            sq[:, 0:w],
                            start=(dt == 0),
                            stop=(dt == 1),
                            skip_group_check=True,
                        )
                    ss_sb = small.tile([1, 512], f32, tag="ss_sb")
                    nc.scalar.copy(ss_sb[:, 0:w], ss_ps[:, 0:w])
                    nc.sync.dma_start(
                        out=cc_in[2048 + moff : 2048 + moff + w].rearrange(
                            "(a n) -> a n", a=1
                        ),
                        in_=ss_sb[:, 0:w],
                    )
                    # v from the same resident block
                    for ct in range(w // 128):
                        mtg = moff // 128 + ct
                        psv = vps.tile([128, 256], f32, tag="v")
                        for et in range(8):
                            nc.tensor.matmul(
                                psv,
                                blk[:, et, ct * 128 : ct * 128 + 128],
                                wv_sb[:, et, :],
                                start=(et == 0),
                                stop=(et == 7),
                            )
                        nc.vector.tensor_copy(
                            v_sb[:, mtg, :, 0:C],
                            psv.rearrange("p (h c) -> p h c", c=C),
                        )

                # ---- single fused AllReduce (q sumsq | k sumsq) ----
                nc.gpsimd.collective_compute(
                    "AllReduce",
                    mybir.AluOpType.add,
                    replica_groups=[[0, 1, 2, 3], [4, 5, 6, 7]],
                    ins=[cc_in.opt()],
                    outs=[cc_out.opt()],
                )

                # wo weight load (DMA only; hides under the AllReduce)
                wo_sb = wts.tile([128, 2, D], f32r, tag="w")
                for dc in range(2):
                    nc.sync.dma_start(
                        out=wo_sb[:, dc, :], in_=woT_d[dc * 128 : dc * 128 + 128, :]
                    )

                # ---- warm burst: dependency-free matmuls spanning the
                # AllReduce latency so the PE HAM clock-gate stays at 8/8
                # when attention starts (a PE-idle window here would drop it
                # to 4/8 and the attention stream would start cold) ----
                warm = projps.tile([128, 512], f32, tag="proj")
                for i in range(WARM_MM):
                    nc.tensor.matmul(
                        warm,
                        kT[:, 0, 0:128],
                        kT[:, 0, 512:1024],
                        start=(i == 0),
                        stop=(i == WARM_MM - 1),
                        skip_group_check=True,
                    )
                warm_sink = small.tile([1, 512], f32, tag="rd")
                nc.vector.tensor_copy(warm_sink, warm[0:1, :])

                # ---- rstd_q: row layout [1, 2048] for the bcast outer-products ----
                ssq_row = singles.tile([1, 2048], f32)
                nc.sync.dma_start(
                    out=ssq_row, in_=cc_out[0:2048].rearrange("(a n) -> a n", a=1)
                )
                nc.scalar.activation(
                    ssq_row, ssq_row, AF.Sqrt, bias=eps_t[0:1, :], scale=invd_t[0:1, :]
                )
                rs_row = singles.tile([1, 2048], f32)
                nc.vector.reciprocal_approx_fast(out=rs_row, in_=ssq_row)
                # q finalize: qT[d, n] *= rstd_q[n] via ones outer-product bcast
                for nb in range(4):
                    bcq = projps.tile([128, 512], f32, tag="proj")
                    nc.tensor.matmul(
                        bcq,
                        ones1x128,
                        rs_row[:, nb * 512 : nb * 512 + 512],
                        start=True,
                        stop=True,
                    )
                    for dt in range(2):
                        nc.vector.tensor_mul(qT[:, dt, nb, :], qT[:, dt, nb, :], bcq)

                # ---- rstd_k: [128, n_mt] lane-parallel; feeds exp scale ----
                ss128 = small.tile([128, n_mt], f32, tag="ssk")
                nc.sync.dma_start(
                    out=ss128, in_=cc_out[2048 : 2048 + M].rearrange("(t p) -> p t", p=128)
                )
                stdk = small.tile([128, n_mt], f32, tag="stdk")
                nc.scalar.activation(stdk, ss128, AF.Sqrt, bias=eps_t, scale=invd_t)
                rstdk = singles.tile([128, n_mt], f32)
                nc.vector.reciprocal_approx_fast(out=rstdk, in_=stdk)

            # ---- attention: (nbp, h) passes, mt pipelined depth-2 ----
            # PSUM banks: s2 2x2 + o2 2 + dummy 1 + outproj 1 = 8.
            with (
                tc.tile_pool(name="sps", bufs=2, space="PSUM") as spool,
                tc.tile_pool(name="ops", bufs=1, space="PSUM") as opool,
                tc.tile_pool(name="dmy", bufs=1, space="PSUM") as dmypool,
                tc.tile_pool(name="fps", bufs=1, space="PSUM") as fpool,
            ):
                dum = dmypool.tile([128, 512], f32, tag="dum")

                def emit_dummy():
                    nc.tensor.matmul(
                        dum, kT[:, 0, 0:128], kT[:, 0, 512:1024],
                        start=True, stop=True, skip_group_check=True,
                    )

                def make_outproj(nbp):
                    """one closure per (nb, ot) psum tile: 2 matmuls + copy + dma"""
                    fns = []
                    for nb in (2 * nbp, 2 * nbp + 1):
                        for ot in range(8):
                            def f(nb=nb, ot=ot):
                                ps = fpool.tile([128, 512], f32, tag="out")
                                for dc in range(2):
                                    nc.tensor.matmul(
                                        ps,
                                        wo_sb[:, dc, ot * 128 : ot * 128 + 128],
                                        xT[:, dc, nb, :],
                                        start=(dc == 0),
                                        stop=(dc == 1),
                                    )
                                out_sb = outbuf.tile([128, 512], f32, tag="osb")
                                nc.vector.tensor_copy(out_sb, ps)
                                nc.sync.dma_start(
                                    out=outT_d[
                                        ot * 128 : ot * 128 + 128,
                                        nb * 512 : nb * 512 + 512,
                                    ],
                                    in_=out_sb,
                                )
                            fns.append(f)
                    return fns

                def emit_normalize(state):
                    """bc outer-products + muls for a pass whose DVE recips are
                    done by now (emitted one pass late to keep PE gapless)."""
                    hh, nbp_, oo_sb, rds_ = state
                    ddt, ooff = hh // 2, (hh % 2) * C
                    for i, nb in enumerate((2 * nbp_, 2 * nbp_ + 1)):
                        bc = spool.tile([128, 2, 512], f32, tag="s2")
                        nc.tensor.matmul(
                            bc[0:C, 0, :], ones1x64, rds_[i], start=True, stop=True
                        )
                        nc.vector.tensor_mul(
                            xT[ooff : ooff + C, ddt, nb, :],
                            oo_sb[0:C, i, :],
                            bc[0:C, 0, :],
                        )

                filler = []
                prev = None
                for nbp in range(2):
                    nbs = (2 * nbp, 2 * nbp + 1)
                    for h in range(4):
                        dt, off = h // 2, (h % 2) * C
                        o2 = opool.tile([C + 1, 2, 512], f32, tag="o2")
                        s2s, p2s = {}, {}

                        def emit_qk(mt):
                            s2 = spool.tile([128, 2, 512], f32, tag="s2")
                            kT_lhs = kT[off : off + C, dt, mt * 128 : mt * 128 + 128]
                            for i, nb in enumerate(nbs):
                                nc.tensor.matmul(
                                    s2[:, i, :],
                                    kT_lhs,
                                    qT[off : off + C, dt, nb, :],
                                    start=True,
                                    stop=True,
                                )
                            s2s[mt] = s2

                        def emit_exp(mt):
                            p2 = ppool.tile([128, 2, 512], f32r, tag="p")
                            nc.scalar.activation(
                                p2, s2s.pop(mt), AF.Exp,
                                bias=mb_sb[:, mt : mt + 1],
                                scale=rstdk[:, mt : mt + 1],
                            )
                            p2s[mt] = p2

                        def emit_pv(mt):
                            p2 = p2s.pop(mt)
                            for i in range(2):
                                nc.tensor.matmul(
                                    o2[:, i, :],
                                    v_sb[:, mt, h, :],
                                    p2[:, i, :],
                                    start=(mt == 0),
                                    stop=(mt == n_mt - 1),
                                    skip_group_check=True,
                                )

                        emit_qk(0)
                        emit_qk(1)
                        emit_exp(0)
                        for mt in range(1, n_mt):
                            emit_pv(mt - 1)
                            if mt + 1 < n_mt:
                                emit_qk(mt + 1)
                            if filler:
                                filler.pop(0)()
                            else:
                                emit_dummy()
                            emit_exp(mt)
                        emit_pv(n_mt - 1)

                        # drain: free o2 with one DVE copy; recips for the
                        # denominators; normalize the PREVIOUS pass (its
                        # recips are certainly done by now)
                        o_sb = obuf.tile([C + 1, 2, 512], f32, tag="osb")
                        nc.vector.tensor_copy(o_sb, o2)
                        rds = []
                        for i in range(2):
                            den = rdp.tile([1, 512], f32, tag="den")
                            nc.vector.tensor_copy(den, o_sb[C : C + 1, i, :])
                            rd = rdp.tile([1, 512], f32, tag="rd")
                            nc.vector.reciprocal_approx_fast(out=rd, in_=den)
                            rds.append(rd)
                        if prev is not None:
                            emit_normalize(prev)
                            if prev[1] == 0 and prev[0] == 3:
                                # nbp=0 fully normalized -> its out-projection
                                # becomes PE filler for the remaining passes
                                filler.extend(make_outproj(0))
                        prev = (h, nbp, o_sb, rds)

                emit_normalize(prev)
                # tail: whatever filler wasn't consumed + second-half out proj
                for f in filler:
                    f()
                for f in make_outproj(1):
                    f()
                sink = small.tile([1, 512], f32, tag="ss_sb")
                nc.vector.tensor_copy(sink, dum[0:1, :])

    nc.finalize()
    return nc


_NC_CACHE = {}


def _get_nc(n_mt: int):
    if n_mt not in _NC_CACHE:
        _NC_CACHE[n_mt] = build(n_mt)
    return _NC_CACHE[n_mt]


def n_mt_for(mask) -> int:
    mask = np.asarray(mask)
    cnt = int(max((mask[b] != 0).sum() for b in range(B)))
    return max(1, (cnt + 127) // 128)


def make_in_maps(querys, key_feats, mask, Wq, Wk, Wv, gq, gk, Wo, bo, n_mt):
    M = n_mt * 128
    querys = np.asarray(querys, dtype=np.float32)
    key_feats = np.asarray(key_feats, dtype=np.float32)
    mask = np.asarray(mask)
    gq = np.asarray(gq, dtype=np.float32)
    gk = np.asarray(gk, dtype=np.float32)

    gsq_full = gq * np.float32(SCALE)  # folded into Wq rows
    gsk_full = gk.astype(np.float32)  # folded into Wk rows
    Wq_f = np.asarray(Wq, dtype=np.float32) * gsq_full[:, None]
    Wk_f = np.asarray(Wk, dtype=np.float32) * gsk_full[:, None]

    qT = [round_f32r(querys[b].T) for b in range(B)]
    kfT, mb = [], []
    for b in range(B):
        idx = np.flatnonzero(mask[b] != 0)
        cnt = len(idx)
        kc = np.zeros((M, E), np.float32)
        kc[:cnt] = key_feats[b][idx]
        kfT.append(round_f32r(kc.T))
        mbv = np.full((M,), NEG, dtype=np.float32)
        mbv[:cnt] = 0.0
        mb.append(mbv.reshape(n_mt, 128))

    wqT, wkT, wvT, woT, ig2q, ig2k = [], [], [], [], [], []
    for j in range(4):
        dsl = slice(j * DS, (j + 1) * DS)
        wqT.append(round_f32r(Wq_f[dsl].T))
        wkT.append(round_f32r(Wk_f[dsl].T))
        wvT.append(round_f32r(np.asarray(Wv)[dsl].T))
        woT.append(round_f32r(np.asarray(Wo)[:, dsl].T))
        # sumsq compensation: raw sumsq = sum_d (q'_d)^2 / gs_d^2
        ig2q.append(round_f32r((1.0 / gsq_full[dsl] ** 2).reshape(2, 128)))
        ig2k.append(round_f32r((1.0 / gsk_full[dsl] ** 2).reshape(2, 128)))

    in_maps = []
    for cid in range(NCORES):
        b, j = cid // 4, cid % 4
        in_maps.append(
            {
                "qT": qT[b],
                "kfT": kfT[b],
                "wqT": wqT[j],
                "wkT": wkT[j],
                "wvT": wvT[j],
                "woT": woT[j],
                "ig2q": ig2q[j],
                "ig2k": ig2k[j],
                "mbias": mb[b],
            }
        )
    return in_maps


def assemble(results, bo):
    bo = np.asarray(bo, dtype=np.float32)
    out = np.zeros((B, N, D), dtype=np.float32)
    for cid in range(NCORES):
        b = cid // 4
        out[b] += results[cid]["outT"].T
    out += bo
    return out


def kernel(querys, key_feats, mask, Wq, Wk, Wv, gq, gk, Wo, bo):
    n_mt = n_mt_for(mask)
    nc = _get_nc(n_mt)
    in_maps = make_in_maps(querys, key_feats, mask, Wq, Wk, Wv, gq, gk, Wo, bo, n_mt)
    res = run_bass_kernel_spmd(nc, in_maps, list(range(NCORES)))
    return assemble(res.results, bo)


# revision 9
# speedup vs baseline: 1.4487x; 1.1754x over previous
"""CrossAttention Trainium2 kernel (8 NeuronCores).

Reference computation (B=2, N=M=2048, D=1024, H=16, C=64):
    q = rmsnorm(querys @ Wq.T, gq) * C**-0.5       [B,N,D]
    k = rmsnorm(key_feats @ Wk.T, gk)              [B,M,D]
    v = key_feats @ Wv.T                           [B,M,D]
    attn = softmax(mask(q @ k.T per head))         [B,H,N,M]
    out = (attn @ v per head, concat) @ Wo.T + bo  [B,N,D]

Sharding: core = b*4 + j (b in {0,1}; j in {0..3} owns heads 4j..4j+3 = a
256-wide slice of D). Host pre-transposes inputs/weights, folds gq*scale /
gk into Wq / Wk rows, and pre-rounds everything to f32r (fp32 with 11-bit
mantissa -> full PE rate).

v2 structural changes vs v1:
  - Mask compaction: rows with mask==0 contribute exp(-inf)=0 to both the
    softmax denominator and PV, so the host gathers only the valid kf
    columns (per batch), pads to a multiple of 128, and the kernel runs
    with M_pad ~= 1152 instead of 2048.  All M-side work (k/v projection,
    QK, exp, PV, kf DMA) shrinks ~2x.  Padding columns carry bias -1e30
    into the exp -> contribute exactly 0.
  - k and v projections fused over a single kfT stream (halves kf DMA).
  - One fused AllReduce carries both q and k partial sum-of-squares
    (2048 + M_pad floats); its ~27us mesh latency is bridged by a long
    dependency-free dummy-matmul burst that keeps the PE HAM clock-gate
    warm (K=8/8) into attention.
  - Attention is software-pipelined depth-2 per (nbp, h) pass: PE order is
    ... PV(mt-1), QK(mt+1), filler, PV(mt) ... so the PE never stalls on
    the ACT exp (v1 stalled ~0.4us every mt, which kept HAM at K=4/8 =
    1.2 GHz for the whole 314us attention phase).  ACT exp (~1.2us/mt) is
    the pace-setter; PE real work is ~1.0us/mt, padded by a dummy matmul
    (first n-half) or an out-projection matmul pair (second n-half).
  - The out projection (partial over this core's d-slice) is interleaved
    into attention as filler work; the host sums 4 partials per b and
    adds bo.
"""

import numpy as np

import concourse.tile as tile
from concourse import bacc, mybir
from concourse.bass_utils import run_bass_kernel_spmd

B, N, M_FULL, D, H = 2, 2048, 2048, 1024, 16
C = D // H  # 64, head dim
E = D  # input feature dim
EPS = 1e-6
SCALE = C ** (-0.5)
DS = D // 4  # 256, per-core d-slice
NCORES = 8

f32 = mybir.dt.float32
f32r = mybir.dt.float32r
AF = mybir.ActivationFunctionType

NEG = -1e30
WARM_MM = 150  # dummy matmuls bridging the AllReduce latency (~42us @ 2.4GHz)


def round_f32r(x: np.ndarray) -> np.ndarray:
    b = np.ascontiguousarray(x, dtype=np.float32).view(np.uint32)
    b = (b + 0x800) & np.uint32(0xFFFFF000)
    return b.view(np.float32)


def build(n_mt: int):
    M = n_mt * 128
    mblocks = []
    off = 0
    while off < M:
        w = min(512, M - off)
        mblocks.append((off, w))
        off += w

    nc = bacc.Bacc(None, target_bir_lowering=False)

    qT_d = nc.declare_dram_parameter("qT", [E, N], f32r, isOutput=False)
    kfT_d = nc.declare_dram_parameter("kfT", [E, M], f32r, isOutput=False)
    wqT_d = nc.declare_dram_parameter("wqT", [E, DS], f32r, isOutput=False)
    wkT_d = nc.declare_dram_parameter("wkT", [E, DS], f32r, isOutput=False)
    wvT_d = nc.declare_dram_parameter("wvT", [E, DS], f32r, isOutput=False)
    woT_d = nc.declare_dram_parameter("woT", [DS, D], f32r, isOutput=False)
    ig2q_d = nc.declare_dram_parameter("ig2q", [2, 128], f32r, isOutput=False)
    ig2k_d = nc.declare_dram_parameter("ig2k", [2, 128], f32r, isOutput=False)
    mb_d = nc.declare_dram_parameter("mbias", [n_mt, 128], f32, isOutput=False)
    outT_d = nc.declare_dram_parameter("outT", [D, N], f32, isOutput=True)

    with (
        nc.allow_low_precision(reason="f32r matmul operands by design; fp32 PSUM"),
        tile.TileContext(nc) as tc,
    ):
        with (
            tc.tile_pool(name="singles", bufs=1) as singles,
            tc.tile_pool(name="wts", bufs=3) as wts,
            tc.tile_pool(name="blk", bufs=2) as blkpool,
            tc.tile_pool(name="sq", bufs=2) as sqpool,
            tc.tile_pool(name="psb", bufs=3) as ppool,
            tc.tile_pool(name="obuf", bufs=2) as obuf,
            tc.tile_pool(name="osb2", bufs=2) as outbuf,
            tc.tile_pool(name="rdp", bufs=6) as rdp,
            tc.tile_pool(name="small", bufs=2) as small,
            tc.tile_pool(name="dram", bufs=1, space="DRAM") as dram,
        ):
            # ---- constants / small inputs ----
            ones1x64 = singles.tile([1, 64], f32)
            nc.vector.memset(ones1x64, 1.0)
            ones1x128 = singles.tile([1, 128], f32)
            nc.vector.memset(ones1x128, 1.0)
            onesv = singles.tile([128, n_mt * 4], f32)
            nc.vector.memset(onesv, 1.0)
            eps_t = singles.tile([128, 1], f32)
            nc.vector.memset(eps_t, EPS)
            invd_t = singles.tile([128, 1], f32)
            nc.vector.memset(invd_t, 1.0 / D)
            ig2q_sb = singles.tile([128, 2], f32r)
            nc.sync.dma_start(out=ig2q_sb, in_=ig2q_d.rearrange("t p -> p t"))
            ig2k_sb = singles.tile([128, 2], f32r)
            nc.sync.dma_start(out=ig2k_sb, in_=ig2k_d.rearrange("t p -> p t"))
            mb_sb = singles.tile([128, n_mt], f32)
            nc.sync.dma_start(out=mb_sb, in_=mb_d.rearrange("t p -> p t"))

            # weights: wq, wk, wv upfront; wo reuses wq's slot after q proj
            wq_sb = wts.tile([128, 8, DS], f32r, tag="w")
            wk_sb = wts.tile([128, 8, DS], f32r, tag="w")
            wv_sb = wts.tile([128, 8, DS], f32r, tag="w")
            for et in range(8):
                nc.sync.dma_start(out=wq_sb[:, et, :], in_=wqT_d[et * 128 : et * 128 + 128, :])
                nc.scalar.dma_start(out=wk_sb[:, et, :], in_=wkT_d[et * 128 : et * 128 + 128, :])
                nc.gpsimd.dma_start(out=wv_sb[:, et, :], in_=wvT_d[et * 128 : et * 128 + 128, :])

            # ---- persistent activations ----
            qT = singles.tile([128, 2, 4, 512], f32r)  # [p, dt, nb, n]
            kT = singles.tile([128, 2, M], f32r)  # [p, dt, m]
            v_sb = singles.tile([128, n_mt, 4, C + 1], f32r)  # [m_p, mt, h, c|ones]
            xT = singles.tile([128, 2, 4, 512], f32r)  # [p, dt, nb, n]
            nc.vector.tensor_copy(
                v_sb[:, :, :, C], onesv.rearrange("p (a b) -> p a b", a=n_mt)
            )

            cc_in = dram.tile([2048 + M], f32)
            cc_out = dram.tile([2048 + M], f32)
            rsq_dram = dram.tile([2048], f32)

            with (
                tc.tile_pool(name="projps", bufs=2, space="PSUM") as projps,
                tc.tile_pool(name="vps", bufs=2, space="PSUM") as vps,
                tc.tile_pool(name="ssps", bufs=2, space="PSUM") as ssps,
            ):
                # ---- q projection: qT[dt, nb] = Wq'^T-slice @ q-block ----
                for nb in range(4):
                    blk = blkpool.tile([128, 8, 512], f32r, tag="blk")
                    for et in range(8):
                        nc.sync.dma_start(
                            out=blk[:, et, :],
                            in_=qT_d[et * 128 : et * 128 + 128, nb * 512 : nb * 512 + 512],
                        )
                    ss_ps = ssps.tile([1, 512], f32, tag="ss")
                    for dt in range(2):
                        ps = projps.tile([128, 512], f32, tag="proj")
                        for et in range(8):
                            nc.tensor.matmul(
                                ps,
                                wq_sb[:, et, dt * 128 : dt * 128 + 128],
                                blk[:, et, :],
                                start=(et == 0),
                                stop=(et == 7),
                            )
                        nc.vector.tensor_copy(qT[:, dt, nb, :], ps)
                        sq = sqpool.tile([128, 512], f32r, tag="sq")
                        nc.vector.tensor_mul(sq, qT[:, dt, nb, :], qT[:, dt, nb, :])
                        nc.tensor.matmul(
                            ss_ps,
                            ig2q_sb[:, dt : dt + 1],
                            sq,
                            start=(dt == 0),
                            stop=(dt == 1),
                            skip_group_check=True,
                        )
                    ss_sb = small.tile([1, 512], f32, tag="ss_sb")
                    nc.scalar.copy(ss_sb, ss_ps)
                    nc.sync.dma_start(
                        out=cc_in[nb * 512 : nb * 512 + 512].rearrange(
                            "(a n) -> a n", a=1
                        ),
                        in_=ss_sb,
                    )

                # ---- fused k+v projection over one kfT stream ----
                for moff, w in mblocks:
                    blk = blkpool.tile([128, 8, 512], f32r, tag="blk")
                    for et in range(8):
                        nc.scalar.dma_start(
                            out=blk[:, et, 0:w],
                            in_=kfT_d[et * 128 : et * 128 + 128, moff : moff + w],
                        )
                    ss_ps = ssps.tile([1, 512], f32, tag="ss")
                    for dt in range(2):
                        ps = projps.tile([128, 512], f32, tag="proj")
                        for et in range(8):
                            nc.tensor.matmul(
                                ps[:, 0:w],
                                wk_sb[:, et, dt * 128 : dt * 128 + 128],
                                blk[:, et, 0:w],
                                start=(et == 0),
                                stop=(et == 7),
                            )
                        nc.vector.tensor_copy(kT[:, dt, moff : moff + w], ps[:, 0:w])
                        sq = sqpool.tile([128, 512], f32r, tag="sq")
                        nc.vector.tensor_mul(
                            sq[:, 0:w],
                            kT[:, dt, moff : moff + w],
                            kT[:, dt, moff : moff + w],
                        )
                        nc.tensor.matmul(
                            ss_ps[:, 0:w],
                            ig2k_sb[:, dt : dt + 1],
                            sq[:, 0:w],
                            start=(dt == 0),
                            stop=(dt == 1),
                            skip_group_check=True,
                        )
                    ss_sb = small.tile([1, 512], f32, tag="ss_sb")
                    nc.scalar.copy(ss_sb[:, 0:w], ss_ps[:, 0:w])
                    nc.sync.dma_start(
                        out=cc_in[2048 + moff : 2048 + moff + w].rearrange(
                            "(a n) -> a n", a=1
                        ),
                        in_=ss_sb[:, 0:w],
                    )
                    # v from the same resident block
                    for ct in range(w // 128):
                        mtg = moff // 128 + ct
                        psv = vps.tile([128, 256], f32, tag="v")
                        for et in range(8):
                            nc.tensor.matmul(
                                psv,
                                blk[:, et, ct * 128 : ct * 128 + 128],
                                wv_sb[:, et, :],
                                start=(et == 0),
                                stop=(et == 7),
                            )
                        nc.vector.tensor_copy(
                            v_sb[:, mtg, :, 0:C],
                            psv.rearrange("p (h c) -> p h c", c=C),
                        )

                # ---- single fused AllReduce (q sumsq | k sumsq) ----
                nc.gpsimd.collective_compute(
                    "AllReduce",
                    mybir.AluOpType.add,
                    replica_groups=[[0, 1, 2, 3], [4, 5, 6, 7]],
                    ins=[cc_in.opt()],
                    outs=[cc_out.opt()],
                )

                # wo weight load (DMA only; hides under the AllReduce)
                wo_sb = wts.tile([128, 2, D], f32r, tag="w")
                for dc in range(2):
                    nc.sync.dma_start(
                        out=wo_sb[:, dc, :], in_=woT_d[dc * 128 : dc * 128 + 128, :]
                    )

                # ---- warm burst: dependency-free matmuls spanning the
                # AllReduce latency so the PE HAM clock-gate stays at 8/8
                # when attention starts (a PE-idle window here would drop it
                # to 4/8 and the attention stream would start cold) ----
                warm = projps.tile([128, 512], f32, tag="proj")
                for i in range(WARM_MM):
                    nc.tensor.matmul(
                        warm,
                        kT[:, 0, 0:128],
                        kT[:, 0, 512:1024],
                        start=(i == 0),
                        stop=(i == WARM_MM - 1),
                        skip_group_check=True,
                    )
                warm_sink = small.tile([1, 512], f32, tag="rd")
                nc.vector.tensor_copy(warm_sink, warm[0:1, :])

                # ---- rstd_k: [128, n_mt] lane-parallel; feeds exp scale ----
                ss128 = small.tile([128, n_mt], f32, tag="ssk")
                nc.sync.dma_start(
                    out=ss128, in_=cc_out[2048 : 2048 + M].rearrange("(t p) -> p t", p=128)
                )
                stdk = small.tile([128, n_mt], f32, tag="stdk")
                nc.scalar.activation(stdk, ss128, AF.Sqrt, bias=eps_t, scale=invd_t)
                rstdk = singles.tile([128, n_mt], f32)
                nc.vector.reciprocal_approx_fast(out=rstdk, in_=stdk)

                # ---- rstd_q: lane-parallel [128, 16], then a DRAM bounce to
                # the [1, 2048] row layout the bcast outer-product wants
                # (single-lane sqrt/recip on [1, 2048] costs ~5us; this ~1us)
                ssq128 = small.tile([128, 16], f32, tag="ssq")
                nc.sync.dma_start(
                    out=ssq128, in_=cc_out[0:2048].rearrange("(t p) -> p t", p=128)
                )
                stdq = small.tile([128, 16], f32, tag="stdq")
                nc.scalar.activation(stdq, ssq128, AF.Sqrt, bias=eps_t, scale=invd_t)
                rsq128 = small.tile([128, 16], f32, tag="rsq")
                nc.vector.reciprocal_approx_fast(out=rsq128, in_=stdq)
                nc.sync.dma_start(
                    out=rsq_dram.rearrange("(t p) -> p t", p=128), in_=rsq128
                )
                rs_row = singles.tile([1, 2048], f32)
                nc.sync.dma_start(
                    out=rs_row, in_=rsq_dram.rearrange("(a n) -> a n", a=1)
                )
                # q finalize: qT[d, n] *= rstd_q[n] via ones outer-product bcast
                for nb in range(4):
                    bcq = projps.tile([128, 512], f32, tag="proj")
                    nc.tensor.matmul(
                        bcq,
                        ones1x128,
                        rs_row[:, nb * 512 : nb * 512 + 512],
                        start=True,
                        stop=True,
                    )
                    for dt in range(2):
                        nc.vector.tensor_mul(qT[:, dt, nb, :], qT[:, dt, nb, :], bcq)

            # ---- attention: flat stream over g = (pass, mt), pipelined
            # depth-2 ACROSS pass boundaries.  PE order:
            #   ... PV(g-1), QK(g+1), filler, PV(g), QK(g+2), ...
            # ACT order: exp(0), exp(1), ...  The PE never waits on an exp
            # (QK(g+1)'s s2 slot was freed by exp(g-1), one full exp ago) so
            # the HAM clock-gate stays warm.
            # PSUM banks: s2 2x2 + o2 2 + filler/bc/dummy 2x1 = 8.
            with (
                tc.tile_pool(name="sps", bufs=2, space="PSUM") as spool,
                tc.tile_pool(name="ops", bufs=1, space="PSUM") as opool,
                tc.tile_pool(name="fps", bufs=2, space="PSUM") as fpool,
            ):
                passes = [(nbp, h) for nbp in range(2) for h in range(4)]
                G = len(passes) * n_mt
                s2g, p2g, o2cur = {}, {}, {}
                filler = []
                prev = [None]

                def emit_dummy():
                    dum = fpool.tile([128, 512], f32, tag="f")
                    nc.tensor.matmul(
                        dum, kT[:, 0, 0:128], kT[:, 0, 512:1024],
                        start=True, stop=True, skip_group_check=True,
                    )

                def make_outproj(nbp):
                    """one closure per (nb, ot) psum tile: 2 matmuls + copy + dma"""
                    fns = []
                    for nb in (2 * nbp, 2 * nbp + 1):
                        for ot in range(8):
                            def f(nb=nb, ot=ot):
                                ps = fpool.tile([128, 512], f32, tag="f")
                                for dc in range(2):
                                    nc.tensor.matmul(
                                        ps,
                                        wo_sb[:, dc, ot * 128 : ot * 128 + 128],
                                        xT[:, dc, nb, :],
                                        start=(dc == 0),
                                        stop=(dc == 1),
                                    )
                                out_sb = outbuf.tile([128, 512], f32, tag="osb")
                                nc.vector.tensor_copy(out_sb, ps)
                                nc.sync.dma_start(
                                    out=outT_d[
                                        ot * 128 : ot * 128 + 128,
                                        nb * 512 : nb * 512 + 512,
                                    ],
                                    in_=out_sb,
                                )
                            fns.append(f)
                    return fns

                def emit_normalize(state):
                    """bc outer-products + muls for a pass whose DVE recips are
                    done by now (emitted one pass late to keep PE gapless)."""
                    hh, nbp_, oo_sb, rds_ = state
                    ddt, ooff = hh // 2, (hh % 2) * C
                    for i, nb in enumerate((2 * nbp_, 2 * nbp_ + 1)):
                        bc = fpool.tile([128, 512], f32, tag="f")
                        nc.tensor.matmul(
                            bc[0:C, :], ones1x64, rds_[i], start=True, stop=True
                        )
                        nc.vector.tensor_mul(
                            xT[ooff : ooff + C, ddt, nb, :],
                            oo_sb[0:C, i, :],
                            bc[0:C, :],
                        )

                def emit_qk(g):
                    (nbp, h), mt = passes[g // n_mt], g % n_mt
                    dt, off = h // 2, (h % 2) * C
                    s2 = spool.tile([128, 2, 512], f32, tag="s2")
                    kT_lhs = kT[off : off + C, dt, mt * 128 : mt * 128 + 128]
                    for i, nb in enumerate((2 * nbp, 2 * nbp + 1)):
                        nc.tensor.matmul(
                            s2[:, i, :],
                            kT_lhs,
                            qT[off : off + C, dt, nb, :],
                            start=True,
                            stop=True,
                        )
                    s2g[g] = s2

                def emit_exp(g):
                    mt = g % n_mt
                    p2 = ppool.tile([128, 2, 512], f32r, tag="p")
                    nc.scalar.activation(
                        p2, s2g.pop(g), AF.Exp,
                        bias=mb_sb[:, mt : mt + 1],
                        scale=rstdk[:, mt : mt + 1],
                    )
                    p2g[g] = p2

                def emit_pv(g):
                    pi, mt = g // n_mt, g % n_mt
                    nbp, h = passes[pi]
                    if mt == 0:
                        o2_t = opool.tile([C + 1, 2, 512], f32, tag="o2")
                        o2cur[pi] = o2_t
                    o2 = o2cur[pi]
                    p2 = p2g.pop(g)
                    for i in range(2):
                        nc.tensor.matmul(
                            o2[:, i, :],
                            v_sb[:, mt, h, :],
                            p2[:, i, :],
                            start=(mt == 0),
                            stop=(mt == n_mt - 1),
                            skip_group_check=True,
                        )
                    if mt == n_mt - 1:
                        # pass drain: free o2 with one DVE copy; recips; the
                        # PREVIOUS pass's normalize (its recips are done)
                        o_sb = obuf.tile([C + 1, 2, 512], f32, tag="osb")
                        nc.vector.tensor_copy(o_sb, o2cur.pop(pi))
                        rds = []
                        for i in range(2):
                            den = rdp.tile([1, 512], f32, tag="den")
                            nc.vector.tensor_copy(den, o_sb[C : C + 1, i, :])
                            rd = rdp.tile([1, 512], f32, tag="rd")
                            nc.vector.reciprocal_approx_fast(out=rd, in_=den)
                            rds.append(rd)
                        if prev[0] is not None:
                            emit_normalize(prev[0])
                            if prev[0][1] == 0 and prev[0][0] == 3:
                                # nbp=0 fully normalized -> its out-projection
                                # becomes PE filler for the remaining passes
                                filler.extend(make_outproj(0))
                        prev[0] = (h, nbp, o_sb, rds)

                emit_qk(0)
                emit_qk(1)
                emit_exp(0)
                for g in range(1, G):
                    emit_pv(g - 1)
                    if g + 1 < G:
                        emit_qk(g + 1)
                    if filler:
                        filler.pop(0)()
                    else:
                        emit_dummy()
                    emit_exp(g)
                emit_pv(G - 1)

                emit_normalize(prev[0])
                # tail: whatever filler wasn't consumed + second-half out proj
                for f in filler:
                    f()
                for f in make_outproj(1):
                    f()

    nc.finalize()
    return nc


_NC_CACHE = {}


def _get_nc(n_mt: int):
    if n_mt not in _NC_CACHE:
        _NC_CACHE[n_mt] = build(n_mt)
    return _NC_CACHE[n_mt]


def n_mt_for(mask) -> int:
    mask = np.asarray(mask)
    cnt = int(max((mask[b] != 0).sum() for b in range(B)))
    return max(1, (cnt + 127) // 128)


def make_in_maps(querys, key_feats, mask, Wq, Wk, Wv, gq, gk, Wo, bo, n_mt):
    M = n_mt * 128
    querys = np.asarray(querys, dtype=np.float32)
    key_feats = np.asarray(key_feats, dtype=np.float32)
    mask = np.asarray(mask)
    gq = np.asarray(gq, dtype=np.float32)
    gk = np.asarray(gk, dtype=np.float32)

    gsq_full = gq * np.float32(SCALE)  # folded into Wq rows
    gsk_full = gk.astype(np.float32)  # folded into Wk rows
    Wq_f = np.asarray(Wq, dtype=np.float32) * gsq_full[:, None]
    Wk_f = np.asarray(Wk, dtype=np.float32) * gsk_full[:, None]

    qT = [round_f32r(querys[b].T) for b in range(B)]
    kfT, mb = [], []
    for b in range(B):
        idx = np.flatnonzero(mask[b] != 0)
        cnt = len(idx)
        kc = np.zeros((M, E), np.float32)
        kc[:cnt] = key_feats[b][idx]
        kfT.append(round_f32r(kc.T))
        mbv = np.full((M,), NEG, dtype=np.float32)
        mbv[:cnt] = 0.0
        mb.append(mbv.reshape(n_mt, 128))

    wqT, wkT, wvT, woT, ig2q, ig2k = [], [], [], [], [], []
    for j in range(4):
        dsl = slice(j * DS, (j + 1) * DS)
        wqT.append(round_f32r(Wq_f[dsl].T))
        wkT.append(round_f32r(Wk_f[dsl].T))
        wvT.append(round_f32r(np.asarray(Wv)[dsl].T))
        woT.append(round_f32r(np.asarray(Wo)[:, dsl].T))
        # sumsq compensation: raw sumsq = sum_d (q'_d)^2 / gs_d^2
        ig2q.append(round_f32r((1.0 / gsq_full[dsl] ** 2).reshape(2, 128)))
        ig2k.append(round_f32r((1.0 / gsk_full[dsl] ** 2).reshape(2, 128)))

    in_maps = []
    for cid in range(NCORES):
        b, j = cid // 4, cid % 4
        in_maps.append(
            {
                "qT": qT[b],
                "kfT": kfT[b],
                "wqT": wqT[j],
                "wkT": wkT[j],
                "wvT": wvT[j],
                "woT": woT[j],
                "ig2q": ig2q[j],
                "ig2k": ig2k[j],
                "mbias": mb[b],
            }
        )
    return in_maps


def assemble(results, bo):
    bo = np.asarray(bo, dtype=np.float32)
    out = np.zeros((B, N, D), dtype=np.float32)
    for cid in range(NCORES):
        b = cid // 4
        out[b] += results[cid]["outT"].T
    out += bo
    return out


def kernel(querys, key_feats, mask, Wq, Wk, Wv, gq, gk, Wo, bo):
    n_mt = n_mt_for(mask)
    nc = _get_nc(n_mt)
    in_maps = make_in_maps(querys, key_feats, mask, Wq, Wk, Wv, gq, gk, Wo, bo, n_mt)
    res = run_bass_kernel_spmd(nc, in_maps, list(range(NCORES)))
    return assemble(res.results, bo)
